# revision 9
# baseline (speedup 1.0000x reference)
"""ConvHex (hex-grid graph conv) Trainium2 Bass kernel.

out[b,o,h] = (Wc@x[b,:,h] + sum_k Wn[:,:,k]@x[b,:,nb[h,k]]*mask) / (1+#valid) + bias

Strategy (8 NeuronCores, data-parallel over batch B=256 -> 32/core):
- x shipped bf16 in two layouts: xr [Hp, 32*64] (rows = all-batch feature
  columns, for the neighbor gather) and xc [16, 128, Hp] (batch-pair tiles
  for the center term; even batch on partitions 0-63, odd on 64-127).
- Neighbor gather: dma_gather(transpose=True) from HBM, 4KB descriptors
  (one descriptor = one hex column for all 32 batches), index tables
  precomputed on host (invalid slots -> zero column Hp-1).
- Matmul: 7 contraction chunks (center + 6 neighbor slots), K=64 each,
  row-tiled pairs: even batch on PE rows 0-63 -> psum_e, odd on rows
  64-127 -> psum_o, accumulated over chunks in PSUM (f32).
- Epilogue: DVE multiply by 1/(1+count) broadcast (precomputed on host),
  f32 out. bias is zero in this problem (asserted; general path adds it).
"""
import os
import numpy as np
import ml_dtypes

B, C_IN, C_OUT, H, K = 256, 64, 128, 1039, 6
NCORES = 8
BL = B // NCORES            # 32 batches per core
NPAIR = BL // 2             # 16
Hp = H + 1                  # 1040; column H (=1039) is the zero pad column
HCS = [384, 384, 272]       # h-chunks (matmul N / psum bank sized)
HC_OFF = [0, 384, 768]
HC_PAD = [384, 384, 384]    # gather num_idxs per chunk (mult of 128)
BF16 = ml_dtypes.bfloat16

TRACE = bool(int(os.environ.get("KERNEL_TRACE", "0")))
LAST_RESULT = None

_CACHE = {}


def _build_program():
    import concourse.mybir as mybir
    import concourse.tile as tile
    from concourse import bacc

    nc = bacc.Bacc(name="convhex")
    dt = mybir.dt
    xr = nc.dram_tensor("xr", [Hp, BL * C_IN], dt.bfloat16, kind="ExternalInput")
    xc = nc.dram_tensor("xc", [NPAIR, 128, Hp], dt.bfloat16, kind="ExternalInput")
    wt = nc.dram_tensor("wt", [128, 7 * 128], dt.bfloat16, kind="ExternalInput")
    inv = nc.dram_tensor("inv", [128, Hp], dt.float32, kind="ExternalInput")
    idxt = nc.dram_tensor("idxt", [128, K, len(HCS), 24], dt.int16,
                          kind="ExternalInput")
    y = nc.dram_tensor("y", [BL, 128, H], dt.bfloat16, kind="ExternalOutput")

    with tile.TileContext(nc) as tc:
        with tc.tile_pool(name="const", bufs=1) as cpool, \
             tc.tile_pool(name="gat", bufs=12) as gpool, \
             tc.tile_pool(name="xcp", bufs=4) as xcpool, \
             tc.tile_pool(name="osb", bufs=2) as opool, \
             tc.tile_pool(name="ps", bufs=2, space="PSUM") as pspool:
            wtile = cpool.tile([128, 7 * 128], dt.bfloat16)
            nc.sync.dma_start(wtile[:], wt[:, :])
            invt = cpool.tile([128, Hp], dt.float32)
            nc.sync.dma_start(invt[:], inv[:, :])
            it = cpool.tile([128, K, len(HCS), 24], dt.int16)
            nc.sync.dma_start(it[:], idxt[:, :, :, :])

            for hci, hn in enumerate(HCS):
                off = HC_OFF[hci]
                npad = HC_PAD[hci]
                # gather all 6 neighbor slots for this h-chunk
                gts = []
                for k in range(K):
                    halves = []
                    for hf in range(2):
                        gt = gpool.tile([128, NPAIR // 2, npad], dt.bfloat16,
                                        tag=f"g{hf}", name=f"g_{hci}_{k}_{hf}")
                        nc.gpsimd.dma_gather(
                            gt[:], xr[:, hf * 1024:(hf + 1) * 1024],
                            it[:, k, hci, 0:npad // 16],
                            num_idxs=npad, num_idxs_reg=npad,
                            elem_size=BL * C_IN // 2, elem_step=BL * C_IN,
                            transpose=True,
                        )
                        halves.append(gt)
                    gts.append(halves)
                for blk in range(NPAIR // 2):
                    ps = []
                    xs = []
                    for j in range(2):
                        p = 2 * blk + j
                        xct = xcpool.tile([128, 384], dt.bfloat16, tag="xc")
                        nc.sync.dma_start(xct[:, 0:hn], xc[p, :, off:off + hn])
                        xs.append(xct)
                        pse = pspool.tile([128, 384], dt.float32, tag=f"pe{j}",
                                          name=f"pse_{hci}_{blk}_{j}")
                        pso = pspool.tile([128, 384], dt.float32, tag=f"po{j}",
                                          name=f"pso_{hci}_{blk}_{j}")
                        ps.append((pse, pso))
                    # chunk-outer: center, then 6 neighbor slots; within a
                    # chunk, 4 matmuls (2 pairs x even/odd row-tiles)
                    for j in range(2):
                        pse, pso = ps[j]
                        nc.tensor.matmul(pse[:, 0:hn], wtile[0:64, 0:128],
                                         xs[j][0:64, 0:hn], start=True, stop=False)
                        nc.tensor.matmul(pso[:, 0:hn], wtile[64:128, 0:128],
                                         xs[j][64:128, 0:hn], start=True, stop=False)
                    for k in range(K):
                        last = k == K - 1
                        wk = wtile[:, (k + 1) * 128:(k + 2) * 128]
                        for j in range(2):
                            p = 2 * blk + j
                            pse, pso = ps[j]
                            gk = gts[k][p // 8]
                            pl = p % 8
                            nc.tensor.matmul(pse[:, 0:hn], wk[0:64, :],
                                             gk[0:64, pl, 0:hn],
                                             start=False, stop=last)
                            nc.tensor.matmul(pso[:, 0:hn], wk[64:128, :],
                                             gk[64:128, pl, 0:hn],
                                             start=False, stop=last)
                    # epilogue: multiply by inv (broadcast along partitions)
                    hv = min(hn, H - off)   # valid output columns
                    for j in range(2):
                        p = 2 * blk + j
                        pse, pso = ps[j]
                        oe = opool.tile([128, 384], dt.bfloat16, tag=f"oe{j}")
                        oo = opool.tile([128, 384], dt.bfloat16, tag=f"oo{j}")
                        nc.vector.tensor_mul(oe[:, 0:hv], pse[:, 0:hv],
                                             invt[:, off:off + hv])
                        nc.vector.tensor_mul(oo[:, 0:hv], pso[:, 0:hv],
                                             invt[:, off:off + hv])
                        nc.sync.dma_start(y[2 * p, :, off:off + hv], oe[:, 0:hv])
                        nc.sync.dma_start(y[2 * p + 1, :, off:off + hv],
                                          oo[:, 0:hv])
    nc.finalize()
    return nc


def _wrap_idx(idx_1d):
    """index list -> [128, n/16] int16 wrapped (pos i at partition i%16, slot i//16)."""
    n = idx_1d.shape[0]
    w = idx_1d.reshape(n // 16, 16).T
    return np.tile(w, (8, 1)).astype(np.int16)


def _host_prep(x, neighbors, weight_center, weight_neighbors, bias):
    x = np.asarray(x, np.float32)
    nb = np.asarray(neighbors)
    wc = np.asarray(weight_center, np.float32)
    wn = np.asarray(weight_neighbors, np.float32)
    bias = np.asarray(bias, np.float32)

    mask = nb >= 0
    counts = mask.sum(1)
    inv = (1.0 / (1.0 + counts)).astype(np.float32)        # [H]
    invp = np.concatenate([inv, np.ones(Hp - H, np.float32)])
    inv_bcast = np.broadcast_to(invp, (128, Hp)).copy()

    safe = np.where(mask, nb, H).astype(np.int16)          # [H, K] -> zero col
    idxt = np.zeros((128, K, len(HCS), 24), np.int16)
    for k in range(K):
        col = np.concatenate([safe[:, k], np.full(Hp - H, H, np.int16)])
        for hci, hn in enumerate(HCS):
            npad = HC_PAD[hci]
            lst = np.full(npad, H, np.int16)
            lst[:hn] = col[HC_OFF[hci]:HC_OFF[hci] + hn]
            idxt[:, k, hci, 0:npad // 16] = _wrap_idx(lst)

    # weights: lhsT [128, 7*128] bf16, chunk c: rows 0-63 = W.T, 64-127 = W.T
    wt = np.zeros((128, 7 * 128), np.float32)
    wt[0:64, 0:128] = wc.T
    wt[64:128, 0:128] = wc.T
    for k in range(K):
        wt[0:64, (k + 1) * 128:(k + 2) * 128] = wn[:, :, k].T
        wt[64:128, (k + 1) * 128:(k + 2) * 128] = wn[:, :, k].T
    wt = wt.astype(BF16)

    xb = x.astype(BF16)                                    # [B, 64, H]
    in_maps = []
    for c in range(NCORES):
        xs = xb[c * BL:(c + 1) * BL]                       # [32, 64, H]
        xrc = np.zeros((Hp, BL, C_IN), BF16)
        xrc[:H] = xs.transpose(2, 0, 1)
        xcc = np.zeros((NPAIR, 128, Hp), BF16)
        xcc[:, 0:64, :H] = xs[0::2]
        xcc[:, 64:128, :H] = xs[1::2]
        in_maps.append({
            "xr": xrc.reshape(Hp, BL * C_IN),
            "xc": xcc,
            "wt": wt,
            "inv": inv_bcast,
            "idxt": idxt,
        })
    return in_maps


def kernel(x, neighbors, weight_center, weight_neighbors, bias):
    global LAST_RESULT
    from concourse.bass_utils import run_bass_kernel_spmd

    if "nc" not in _CACHE:
        _CACHE["nc"] = _build_program()
    nc = _CACHE["nc"]
    in_maps = _host_prep(x, neighbors, weight_center, weight_neighbors, bias)
    res = run_bass_kernel_spmd(nc, in_maps, core_ids=list(range(NCORES)),
                               trace=TRACE)
    LAST_RESULT = res
    out = np.concatenate([r["y"] for r in res.results], axis=0).astype(np.float32)
    b = np.asarray(bias, np.float32)
    if np.any(b != 0.0):
        # reference adds bias after the divide; device epilogue skips it
        out = out + b[None, :, None]
    return np.ascontiguousarray(out)


# revision 10
# speedup vs baseline: 1.0822x; 1.0822x over previous
"""ConvHex (hex-grid graph conv) Trainium2 Bass kernel.

out[b,o,h] = (Wc@x[b,:,h] + sum_k Wn[:,:,k]@x[b,:,nb[h,k]]*mask) / (1+#valid) + bias

Strategy (8 NeuronCores, data-parallel over batch B=256 -> 32/core):
- x shipped bf16 in two layouts: xr [Hp, 32*64] (rows = all-batch feature
  columns, for the neighbor gather) and xc [16, 128, Hp] (batch-pair tiles
  for the center term; even batch on partitions 0-63, odd on 64-127).
- Neighbor gather: dma_gather(transpose=True) from HBM, 4KB descriptors
  (one descriptor = one hex column for all 32 batches), index tables
  precomputed on host (invalid slots -> zero column Hp-1).
- Matmul: 7 contraction chunks (center + 6 neighbor slots), K=64 each,
  row-tiled pairs: even batch on PE rows 0-63 -> psum_e, odd on rows
  64-127 -> psum_o, accumulated over chunks in PSUM (f32).
- Epilogue: DVE multiply by 1/(1+count) broadcast (precomputed on host),
  f32 out. bias is zero in this problem (asserted; general path adds it).
"""
import os
import numpy as np
import ml_dtypes

B, C_IN, C_OUT, H, K = 256, 64, 128, 1039, 6
NCORES = 8
BL = B // NCORES            # 32 batches per core
NPAIR = BL // 2             # 16
Hp = H + 1                  # 1040; column H (=1039) is the zero pad column
HCS = [384, 384, 272]       # h-chunks (matmul N / psum bank sized)
HC_OFF = [0, 384, 768]
HC_PAD = [384, 384, 384]    # gather num_idxs per chunk (mult of 128)
BF16 = ml_dtypes.bfloat16

TRACE = bool(int(os.environ.get("KERNEL_TRACE", "0")))
LAST_RESULT = None

_CACHE = {}


def _build_program(active):
    import concourse.mybir as mybir
    import concourse.tile as tile
    from concourse import bacc

    nc = bacc.Bacc(name="convhex")
    dt = mybir.dt
    xr = nc.dram_tensor("xr", [Hp, BL * C_IN], dt.bfloat16, kind="ExternalInput")
    xc = nc.dram_tensor("xc", [NPAIR, 128, Hp], dt.bfloat16, kind="ExternalInput")
    wt = nc.dram_tensor("wt", [128, 7 * 128], dt.bfloat16, kind="ExternalInput")
    inv = nc.dram_tensor("inv", [128, Hp], dt.float32, kind="ExternalInput")
    idxt = nc.dram_tensor("idxt", [128, K, len(HCS), 24], dt.int16,
                          kind="ExternalInput")
    y = nc.dram_tensor("y", [BL, 128, H], dt.bfloat16, kind="ExternalOutput")

    with tile.TileContext(nc) as tc:
        with tc.tile_pool(name="const", bufs=1) as cpool, \
             tc.tile_pool(name="gat", bufs=12) as gpool, \
             tc.tile_pool(name="xcp", bufs=4) as xcpool, \
             tc.tile_pool(name="osb", bufs=2) as opool, \
             tc.tile_pool(name="ps", bufs=2, space="PSUM") as pspool:
            wtile = cpool.tile([128, 7 * 128], dt.bfloat16)
            nc.sync.dma_start(wtile[:], wt[:, :])
            invt = cpool.tile([128, Hp], dt.float32)
            nc.sync.dma_start(invt[:], inv[:, :])
            it = cpool.tile([128, K, len(HCS), 24], dt.int16)
            nc.sync.dma_start(it[:], idxt[:, :, :, :])

            for hci, hn in enumerate(HCS):
                off = HC_OFF[hci]
                npad = HC_PAD[hci]
                # gather all 6 neighbor slots for this h-chunk
                gts = []
                ks_act = [k for k in range(K) if active[k][hci]]
                for k in range(K):
                    if k not in ks_act:
                        gts.append(None)
                        continue
                    halves = []
                    for hf in range(2):
                        gt = gpool.tile([128, NPAIR // 2, npad], dt.bfloat16,
                                        tag=f"g{hf}", name=f"g_{hci}_{k}_{hf}")
                        nc.gpsimd.dma_gather(
                            gt[:], xr[:, hf * 1024:(hf + 1) * 1024],
                            it[:, k, hci, 0:npad // 16],
                            num_idxs=npad, num_idxs_reg=npad,
                            elem_size=BL * C_IN // 2, elem_step=BL * C_IN,
                            transpose=True,
                        )
                        halves.append(gt)
                    gts.append(halves)
                for blk in range(NPAIR // 2):
                    ps = []
                    xs = []
                    for j in range(2):
                        p = 2 * blk + j
                        xct = xcpool.tile([128, 384], dt.bfloat16, tag="xc")
                        nc.sync.dma_start(xct[:, 0:hn], xc[p, :, off:off + hn])
                        xs.append(xct)
                        pse = pspool.tile([128, 384], dt.float32, tag=f"pe{j}",
                                          name=f"pse_{hci}_{blk}_{j}")
                        pso = pspool.tile([128, 384], dt.float32, tag=f"po{j}",
                                          name=f"pso_{hci}_{blk}_{j}")
                        ps.append((pse, pso))
                    # chunk-outer: center, then 6 neighbor slots; within a
                    # chunk, 4 matmuls (2 pairs x even/odd row-tiles)
                    for j in range(2):
                        pse, pso = ps[j]
                        cstop = len(ks_act) == 0
                        nc.tensor.matmul(pse[:, 0:hn], wtile[0:64, 0:128],
                                         xs[j][0:64, 0:hn], start=True, stop=cstop)
                        nc.tensor.matmul(pso[:, 0:hn], wtile[64:128, 0:128],
                                         xs[j][64:128, 0:hn], start=True, stop=cstop)
                    for k in ks_act:
                        last = k == ks_act[-1]
                        wk = wtile[:, (k + 1) * 128:(k + 2) * 128]
                        for j in range(2):
                            p = 2 * blk + j
                            pse, pso = ps[j]
                            gk = gts[k][p // 8]
                            pl = p % 8
                            nc.tensor.matmul(pse[:, 0:hn], wk[0:64, :],
                                             gk[0:64, pl, 0:hn],
                                             start=False, stop=last)
                            nc.tensor.matmul(pso[:, 0:hn], wk[64:128, :],
                                             gk[64:128, pl, 0:hn],
                                             start=False, stop=last)
                    # epilogue: multiply by inv (broadcast along partitions)
                    hv = min(hn, H - off)   # valid output columns
                    for j in range(2):
                        p = 2 * blk + j
                        pse, pso = ps[j]
                        oe = opool.tile([128, 384], dt.bfloat16, tag=f"oe{j}")
                        oo = opool.tile([128, 384], dt.bfloat16, tag=f"oo{j}")
                        nc.vector.tensor_mul(oe[:, 0:hv], pse[:, 0:hv],
                                             invt[:, off:off + hv])
                        nc.vector.tensor_mul(oo[:, 0:hv], pso[:, 0:hv],
                                             invt[:, off:off + hv])
                        nc.sync.dma_start(y[2 * p, :, off:off + hv], oe[:, 0:hv])
                        nc.sync.dma_start(y[2 * p + 1, :, off:off + hv],
                                          oo[:, 0:hv])
    nc.finalize()
    return nc


def _wrap_idx(idx_1d):
    """index list -> [128, n/16] int16 wrapped (pos i at partition i%16, slot i//16)."""
    n = idx_1d.shape[0]
    w = idx_1d.reshape(n // 16, 16).T
    return np.tile(w, (8, 1)).astype(np.int16)


def _host_prep(x, neighbors, weight_center, weight_neighbors, bias):
    x = np.asarray(x, np.float32)
    nb = np.asarray(neighbors)
    wc = np.asarray(weight_center, np.float32)
    wn = np.asarray(weight_neighbors, np.float32)
    bias = np.asarray(bias, np.float32)

    mask = nb >= 0
    counts = mask.sum(1)
    perm = np.argsort(-counts, kind="stable")              # h sorted by count desc
    inv = (1.0 / (1.0 + counts[perm])).astype(np.float32)  # [H] permuted order
    invp = np.concatenate([inv, np.ones(Hp - H, np.float32)])
    inv_bcast = np.broadcast_to(invp, (128, Hp)).copy()

    safe = np.where(mask, nb, H).astype(np.int16)[perm]    # [H, K] rows permuted
    idxt = np.zeros((128, K, len(HCS), 24), np.int16)
    for k in range(K):
        col = np.concatenate([safe[:, k], np.full(Hp - H, H, np.int16)])
        for hci, hn in enumerate(HCS):
            npad = HC_PAD[hci]
            lst = np.full(npad, H, np.int16)
            lst[:hn] = col[HC_OFF[hci]:HC_OFF[hci] + hn]
            idxt[:, k, hci, 0:npad // 16] = _wrap_idx(lst)

    # weights: lhsT [128, 7*128] bf16, chunk c: rows 0-63 = W.T, 64-127 = W.T
    wt = np.zeros((128, 7 * 128), np.float32)
    wt[0:64, 0:128] = wc.T
    wt[64:128, 0:128] = wc.T
    for k in range(K):
        wt[0:64, (k + 1) * 128:(k + 2) * 128] = wn[:, :, k].T
        wt[64:128, (k + 1) * 128:(k + 2) * 128] = wn[:, :, k].T
    wt = wt.astype(BF16)

    xb = x.astype(BF16)                                    # [B, 64, H]
    in_maps = []
    for c in range(NCORES):
        xs = xb[c * BL:(c + 1) * BL]                       # [32, 64, H]
        xrc = np.zeros((Hp, BL, C_IN), BF16)
        xrc[:H] = xs.transpose(2, 0, 1)
        xcc = np.zeros((NPAIR, 128, Hp), BF16)
        xcc[:, 0:64, :H] = xs[0::2][:, :, perm]
        xcc[:, 64:128, :H] = xs[1::2][:, :, perm]
        in_maps.append({
            "xr": xrc.reshape(Hp, BL * C_IN),
            "xc": xcc,
            "wt": wt,
            "inv": inv_bcast,
            "idxt": idxt,
        })
    return in_maps, counts, perm


def kernel(x, neighbors, weight_center, weight_neighbors, bias):
    global LAST_RESULT
    from concourse.bass_utils import run_bass_kernel_spmd

    in_maps, counts, perm = _host_prep(x, neighbors, weight_center,
                                       weight_neighbors, bias)
    nk = tuple(int((counts > k).sum()) for k in range(K))
    active = tuple(tuple(HC_OFF[h] < nk[k] for h in range(len(HCS)))
                   for k in range(K))
    if _CACHE.get("key") != active:
        _CACHE["nc"] = _build_program(active)
        _CACHE["key"] = active
    nc = _CACHE["nc"]
    res = run_bass_kernel_spmd(nc, in_maps, core_ids=list(range(NCORES)),
                               trace=TRACE)
    LAST_RESULT = res
    out = np.concatenate([r["y"] for r in res.results], axis=0).astype(np.float32)
    inv_perm = np.empty_like(perm)
    inv_perm[perm] = np.arange(perm.shape[0])
    out = out[:, :, inv_perm]                   # undo count-sort of h
    b = np.asarray(bias, np.float32)
    if np.any(b != 0.0):
        # reference adds bias after the divide; device epilogue skips it
        out = out + b[None, :, None]
    return np.ascontiguousarray(out)


# revision 11
# speedup vs baseline: 1.0854x; 1.0029x over previous
"""ConvHex (hex-grid graph conv) Trainium2 Bass kernel.

out[b,o,h] = (Wc@x[b,:,h] + sum_k Wn[:,:,k]@x[b,:,nb[h,k]]*mask) / (1+#valid) + bias

Strategy (8 NeuronCores, data-parallel over batch B=256 -> 32/core):
- x shipped bf16 in two layouts: xr [Hp, 32*64] (rows = all-batch feature
  columns, for the neighbor gather) and xc [16, 128, Hp] (batch-pair tiles
  for the center term; even batch on partitions 0-63, odd on 64-127).
- Neighbor gather: dma_gather(transpose=True) from HBM, 4KB descriptors
  (one descriptor = one hex column for all 32 batches), index tables
  precomputed on host (invalid slots -> zero column Hp-1).
- Matmul: 7 contraction chunks (center + 6 neighbor slots), K=64 each,
  row-tiled pairs: even batch on PE rows 0-63 -> psum_e, odd on rows
  64-127 -> psum_o, accumulated over chunks in PSUM (f32).
- Epilogue: DVE multiply by 1/(1+count) broadcast (precomputed on host),
  f32 out. bias is zero in this problem (asserted; general path adds it).
"""
import os
import numpy as np
import ml_dtypes

B, C_IN, C_OUT, H, K = 256, 64, 128, 1039, 6
NCORES = 8
BL = B // NCORES            # 32 batches per core
NPAIR = BL // 2             # 16
Hp = H + 1                  # 1040; column H (=1039) is the zero pad column
HCS = [384, 384, 272]       # h-chunks (matmul N / psum bank sized)
HC_OFF = [0, 384, 768]
HC_PAD = [384, 384, 384]    # gather num_idxs per chunk (mult of 128)
BF16 = ml_dtypes.bfloat16

TRACE = bool(int(os.environ.get("KERNEL_TRACE", "0")))
LAST_RESULT = None

_CACHE = {}


def _build_program(active):
    import concourse.mybir as mybir
    import concourse.tile as tile
    from concourse import bacc

    nc = bacc.Bacc(name="convhex")
    dt = mybir.dt
    xr = nc.dram_tensor("xr", [Hp, BL * C_IN], dt.bfloat16, kind="ExternalInput")
    xc = nc.dram_tensor("xc", [NPAIR, 128, Hp], dt.bfloat16, kind="ExternalInput")
    wt = nc.dram_tensor("wt", [128, 7 * 128], dt.bfloat16, kind="ExternalInput")
    inv = nc.dram_tensor("inv", [128, Hp], dt.float32, kind="ExternalInput")
    idxt = nc.dram_tensor("idxt", [128, K, len(HCS), 24], dt.int16,
                          kind="ExternalInput")
    y = nc.dram_tensor("y", [BL, 128, H], dt.bfloat16, kind="ExternalOutput")

    with tile.TileContext(nc) as tc:
        with tc.tile_pool(name="const", bufs=1) as cpool, \
             tc.tile_pool(name="gat", bufs=13) as gpool, \
             tc.tile_pool(name="xcp", bufs=8) as xcpool, \
             tc.tile_pool(name="osb", bufs=2) as opool, \
             tc.tile_pool(name="ps", bufs=2, space="PSUM") as pspool:
            wtile = cpool.tile([128, 7 * 128], dt.bfloat16)
            nc.sync.dma_start(wtile[:], wt[:, :])
            invt = cpool.tile([128, Hp], dt.float32)
            nc.sync.dma_start(invt[:], inv[:, :])
            it = cpool.tile([128, K, len(HCS), 24], dt.int16)
            nc.sync.dma_start(it[:], idxt[:, :, :, :])

            for hci, hn in enumerate(HCS):
                off = HC_OFF[hci]
                npad = HC_PAD[hci]
                # gather all 6 neighbor slots for this h-chunk
                gts = []
                ks_act = [k for k in range(K) if active[k][hci]]
                for k in range(K):
                    if k not in ks_act:
                        gts.append(None)
                        continue
                    halves = []
                    for hf in range(2):
                        gt = gpool.tile([128, NPAIR // 2, npad], dt.bfloat16,
                                        tag=f"g{hf}", name=f"g_{hci}_{k}_{hf}")
                        nc.gpsimd.dma_gather(
                            gt[:], xr[:, hf * 1024:(hf + 1) * 1024],
                            it[:, k, hci, 0:npad // 16],
                            num_idxs=npad, num_idxs_reg=npad,
                            elem_size=BL * C_IN // 2, elem_step=BL * C_IN,
                            transpose=True,
                        )
                        halves.append(gt)
                    gts.append(halves)
                for blk in range(NPAIR // 2):
                    ps = []
                    xs = []
                    for j in range(2):
                        p = 2 * blk + j
                        xct = xcpool.tile([128, 384], dt.bfloat16, tag="xc")
                        nc.sync.dma_start(xct[:, 0:hn], xc[p, :, off:off + hn])
                        xs.append(xct)
                        pse = pspool.tile([128, 384], dt.float32, tag=f"pe{j}",
                                          name=f"pse_{hci}_{blk}_{j}")
                        pso = pspool.tile([128, 384], dt.float32, tag=f"po{j}",
                                          name=f"pso_{hci}_{blk}_{j}")
                        ps.append((pse, pso))
                    # chunk-outer: center, then 6 neighbor slots; within a
                    # chunk, 4 matmuls (2 pairs x even/odd row-tiles)
                    for j in range(2):
                        pse, pso = ps[j]
                        cstop = len(ks_act) == 0
                        nc.tensor.matmul(pse[:, 0:hn], wtile[0:64, 0:128],
                                         xs[j][0:64, 0:hn], start=True, stop=cstop)
                        nc.tensor.matmul(pso[:, 0:hn], wtile[64:128, 0:128],
                                         xs[j][64:128, 0:hn], start=True, stop=cstop)
                    for k in ks_act:
                        last = k == ks_act[-1]
                        wk = wtile[:, (k + 1) * 128:(k + 2) * 128]
                        for j in range(2):
                            p = 2 * blk + j
                            pse, pso = ps[j]
                            gk = gts[k][p // 8]
                            pl = p % 8
                            nc.tensor.matmul(pse[:, 0:hn], wk[0:64, :],
                                             gk[0:64, pl, 0:hn],
                                             start=False, stop=last)
                            nc.tensor.matmul(pso[:, 0:hn], wk[64:128, :],
                                             gk[64:128, pl, 0:hn],
                                             start=False, stop=last)
                    # epilogue: multiply by inv (broadcast along partitions)
                    hv = min(hn, H - off)   # valid output columns
                    for j in range(2):
                        p = 2 * blk + j
                        pse, pso = ps[j]
                        oe = opool.tile([128, 384], dt.bfloat16, tag=f"oe{j}")
                        oo = opool.tile([128, 384], dt.bfloat16, tag=f"oo{j}")
                        nc.vector.tensor_mul(oe[:, 0:hv], pse[:, 0:hv],
                                             invt[:, off:off + hv])
                        nc.vector.tensor_mul(oo[:, 0:hv], pso[:, 0:hv],
                                             invt[:, off:off + hv])
                        nc.sync.dma_start(y[2 * p, :, off:off + hv], oe[:, 0:hv])
                        nc.sync.dma_start(y[2 * p + 1, :, off:off + hv],
                                          oo[:, 0:hv])
    nc.finalize()
    return nc


def _wrap_idx(idx_1d):
    """index list -> [128, n/16] int16 wrapped (pos i at partition i%16, slot i//16)."""
    n = idx_1d.shape[0]
    w = idx_1d.reshape(n // 16, 16).T
    return np.tile(w, (8, 1)).astype(np.int16)


def _host_prep(x, neighbors, weight_center, weight_neighbors, bias):
    x = np.asarray(x, np.float32)
    nb = np.asarray(neighbors)
    wc = np.asarray(weight_center, np.float32)
    wn = np.asarray(weight_neighbors, np.float32)
    bias = np.asarray(bias, np.float32)

    mask = nb >= 0
    counts = mask.sum(1)
    perm = np.argsort(-counts, kind="stable")              # h sorted by count desc
    inv = (1.0 / (1.0 + counts[perm])).astype(np.float32)  # [H] permuted order
    invp = np.concatenate([inv, np.ones(Hp - H, np.float32)])
    inv_bcast = np.broadcast_to(invp, (128, Hp)).copy()

    safe = np.where(mask, nb, H).astype(np.int16)[perm]    # [H, K] rows permuted
    idxt = np.zeros((128, K, len(HCS), 24), np.int16)
    for k in range(K):
        col = np.concatenate([safe[:, k], np.full(Hp - H, H, np.int16)])
        for hci, hn in enumerate(HCS):
            npad = HC_PAD[hci]
            lst = np.full(npad, H, np.int16)
            lst[:hn] = col[HC_OFF[hci]:HC_OFF[hci] + hn]
            idxt[:, k, hci, 0:npad // 16] = _wrap_idx(lst)

    # weights: lhsT [128, 7*128] bf16, chunk c: rows 0-63 = W.T, 64-127 = W.T
    wt = np.zeros((128, 7 * 128), np.float32)
    wt[0:64, 0:128] = wc.T
    wt[64:128, 0:128] = wc.T
    for k in range(K):
        wt[0:64, (k + 1) * 128:(k + 2) * 128] = wn[:, :, k].T
        wt[64:128, (k + 1) * 128:(k + 2) * 128] = wn[:, :, k].T
    wt = wt.astype(BF16)

    xb = x.astype(BF16)                                    # [B, 64, H]
    in_maps = []
    for c in range(NCORES):
        xs = xb[c * BL:(c + 1) * BL]                       # [32, 64, H]
        xrc = np.zeros((Hp, BL, C_IN), BF16)
        xrc[:H] = xs.transpose(2, 0, 1)
        xcc = np.zeros((NPAIR, 128, Hp), BF16)
        xcc[:, 0:64, :H] = xs[0::2][:, :, perm]
        xcc[:, 64:128, :H] = xs[1::2][:, :, perm]
        in_maps.append({
            "xr": xrc.reshape(Hp, BL * C_IN),
            "xc": xcc,
            "wt": wt,
            "inv": inv_bcast,
            "idxt": idxt,
        })
    return in_maps, counts, perm


def kernel(x, neighbors, weight_center, weight_neighbors, bias):
    global LAST_RESULT
    from concourse.bass_utils import run_bass_kernel_spmd

    in_maps, counts, perm = _host_prep(x, neighbors, weight_center,
                                       weight_neighbors, bias)
    nk = tuple(int((counts > k).sum()) for k in range(K))
    active = tuple(tuple(HC_OFF[h] < nk[k] for h in range(len(HCS)))
                   for k in range(K))
    if _CACHE.get("key") != active:
        _CACHE["nc"] = _build_program(active)
        _CACHE["key"] = active
    nc = _CACHE["nc"]
    res = run_bass_kernel_spmd(nc, in_maps, core_ids=list(range(NCORES)),
                               trace=TRACE)
    LAST_RESULT = res
    out = np.concatenate([r["y"] for r in res.results], axis=0).astype(np.float32)
    inv_perm = np.empty_like(perm)
    inv_perm[perm] = np.arange(perm.shape[0])
    out = out[:, :, inv_perm]                   # undo count-sort of h
    b = np.asarray(bias, np.float32)
    if np.any(b != 0.0):
        # reference adds bias after the divide; device epilogue skips it
        out = out + b[None, :, None]
    return np.ascontiguousarray(out)
